# revision 46
# baseline (speedup 1.0000x reference)
"""Causal segment-masked depthwise conv (K=5) + pointwise conv, 8-core SPMD.

Strategy:
  Host: pack each batch row's segments into a global stream with 4 zeros
  before each segment (plain causal conv on the stream == per-segment
  left-zero-padded conv), split the stream evenly across 8 cores with a
  4-element halo, pre-transpose to [C, stream], and lay the per-block
  slabs out contiguously so every device DMA is one large contiguous run.
  Device: depthwise conv for channel chunks 0-1 as diag-stationary fp32r
  matmuls in PSUM (+ ACT bias copy), chunks 2-3 on DVE via fused
  scalar_tensor_tensor over 1024-wide superblocks; pointwise matmul with
  w_pw^T chunks stationary and dwT moving -> transposed [d, l] PSUM
  output, ACT adds b_pw as per-partition bias, batched store. Host
  transposes back during gather and applies a sparse general-case
  correction for exotic segment overlap patterns (empty for contiguous
  partitions).
"""

import sys

sys.path.insert(0, "/opt/trn_rl_repo")

import numpy as np

B, L, C, K, S = 8, 4096, 512, 5, 8
NCORES = 8
CCH = C // 128          # 4 channel chunks
OUT_ROWS = 4352         # conv outputs for stream cols [4, 4356) of the slab
NBLK = 9                # 8 blocks of 512 + 1 of 256
BLKS = [512] * 8 + [256]
NSB = 4
SBS = [1024] * 4           # DVE superblocks (tail block 8 is done on PE)
PE_CH = 2               # channel chunks 0..PE_CH-1 on PE, rest on DVE
NDV = CCH - PE_CH
PEW = 520               # packed PE block width (blk + 4 halo + pad)
DVW = 1032              # packed DVE superblock width

_cached = {}


def _build_nc():
    import concourse.mybir as mybir
    from concourse import bacc
    from concourse.tile import TileContext

    f32 = mybir.dt.float32
    f32r = mybir.dt.float32r
    Alu = mybir.AluOpType

    nc = bacc.Bacc(num_swdge_queues=2)
    xp_d = nc.declare_dram_parameter("xp", [NBLK, 128, PE_CH, PEW], f32, isOutput=False)
    xv_d = nc.declare_dram_parameter("xv", [NSB, 128, NDV, DVW], f32, isOutput=False)
    xq_d = nc.declare_dram_parameter("xq", [128, NDV, 264], f32, isOutput=False)
    # cst: [0:128]=identity, [128:128+20]=wdiag, then bdw, bpw (4 each)
    cst_d = nc.declare_dram_parameter(
        "cst", [128, 128 + CCH * K + 2 * CCH], f32, isOutput=False
    )
    wpwt_d = nc.declare_dram_parameter(
        "wpwt", [128, CCH, CCH, 128], f32, isOutput=False
    )
    out_d = nc.declare_dram_parameter("out", [NBLK, 128, CCH, 512], f32, isOutput=True)

    with TileContext(nc) as tc:
        with (
            tc.tile_pool(name="consts", bufs=1) as cpool,
            tc.tile_pool(name="xtp", bufs=4) as xtp_pool,
            tc.tile_pool(name="xtv", bufs=3) as xtv_pool,
            tc.tile_pool(name="dwt", bufs=3) as dwt_pool,
            tc.tile_pool(name="dwtv", bufs=3) as dwtv_pool,
            tc.tile_pool(name="outsb", bufs=4) as out_pool,
            tc.tile_pool(name="dwps", bufs=4, space="PSUM") as dw_psum,
            tc.tile_pool(name="outps", bufs=4, space="PSUM") as out_psum,
        ):
            # all small consts in one DMA, first on the sync ring
            cst = cpool.tile([128, 128 + CCH * K + 2 * CCH], f32)
            nc.sync.dma_start(out=cst[:], in_=cst_d[:])
            ident = cst[:, 0:128]
            wdiag_src = cst[:, 128 : 128 + CCH * K]
            bdw = cst[:, 128 + CCH * K : 128 + CCH * K + CCH]
            bpw = cst[:, 128 + CCH * K + CCH : 128 + CCH * K + 2 * CCH]
            wpwt_f = cpool.tile([128, CCH, CCH, 128], f32)
            nc.sync.dma_start(out=wpwt_f[:], in_=wpwt_d[:])

            # diag built by DVE+ACT in parallel (both idle early).
            # First PE_CH*K tiles gate the first matmul; the rest (for the
            # PE-owned tail block) are only needed at the end.
            diag = cpool.tile([128, CCH * K * 128], f32r)
            for u in list(range(PE_CH * K)) + list(range(PE_CH * K, CCH * K)):
                sl = diag[:, u * 128 : (u + 1) * 128]
                wc = wdiag_src[:, u : u + 1]
                if u % 2 == 0:
                    nc.vector.tensor_scalar_mul(sl, ident, wc)
                else:
                    nc.scalar.mul(sl, ident, wc)
            wpwt = cpool.tile([128, CCH, CCH, 128], f32r)
            nc.scalar.copy(wpwt[:], wpwt_f[:])

            # PE warm-up: harmless matmuls on the diag tiles to lift the HAM
            # clock gate before the first real conv block arrives
            warm_ps = dw_psum.tile([128, 512], f32, tag="dwps", name="warm_ps")
            for wi in range(10):
                nc.tensor.matmul(
                    warm_ps[:],
                    lhsT=diag[:, 0:128],
                    rhs=diag[:, 0:512],
                    start=(wi == 0),
                    stop=(wi == 9),
                )

            def load_pe(lb):
                t = xtp_pool.tile([128, PE_CH, PEW], f32r, tag="xtp", name=f"xtp{lb}")
                if lb == 0:
                    # split so the very first conv block's data lands sooner
                    nc.gpsimd.dma_start(out=t[:, 0, :], in_=xp_d[lb, :, 0, :])
                    nc.gpsimd.dma_start(out=t[:, 1, :], in_=xp_d[lb, :, 1, :])
                elif lb == 1:
                    # block 1 via the sync ring (f32) + ACT cast: the SWDGE
                    # ring can't deliver it before the PE finishes block 0
                    tf = xtp_pool.tile(
                        [128, PE_CH, PEW], f32, tag="xtpf", name="xtp1f"
                    )
                    nc.sync.dma_start(out=tf[:], in_=xp_d[lb])
                    nc.scalar.copy(t[:], tf[:])
                else:
                    nc.gpsimd.dma_start(out=t[:], in_=xp_d[lb])
                return t

            def load_dve(sb):
                t = xtv_pool.tile([128, NDV, DVW], f32, tag="xtv", name=f"xtv{sb}")
                eng = nc.scalar if sb % 2 == 0 else nc.sync
                eng.dma_start(out=t[:], in_=xv_d[sb])
                return t

            def conv_dve(sb, xtv):
                sblk = SBS[sb]
                dts = []
                for jj in range(NDV):
                    j = PE_CH + jj
                    dt_ = dwtv_pool.tile(
                        [128, sblk], f32r, tag=f"dwtv{jj}", name=f"dwtv{jj}_{sb}"
                    )
                    nc.vector.tensor_scalar(
                        dt_[:],
                        xtv[:, jj, 0:sblk],
                        wdiag_src[:, j * K : j * K + 1],
                        bdw[:, j : j + 1],
                        op0=Alu.mult,
                        op1=Alu.add,
                    )
                    for k in range(1, K):
                        nc.vector.scalar_tensor_tensor(
                            dt_[:],
                            xtv[:, jj, k : k + sblk],
                            wdiag_src[:, j * K + k : j * K + k + 1],
                            dt_[:],
                            op0=Alu.mult,
                            op1=Alu.add,
                        )
                    dts.append(dt_)
                return dts

            def conv_pe_one(lb, j, rhs_tile, jj):
                blk = BLKS[lb]
                ps = dw_psum.tile([128, blk], f32, tag="dwps", name=f"ps{j}_{lb}")
                for k in range(K):
                    nc.tensor.matmul(
                        ps[:],
                        lhsT=diag[:, (j * K + k) * 128 : (j * K + k + 1) * 128],
                        rhs=rhs_tile[:, jj, k : k + blk],
                        start=(k == 0),
                        stop=(k == K - 1),
                    )
                dt_ = dwt_pool.tile([128, blk], f32r, tag=f"dwt{j}", name=f"dwt{j}_{lb}")
                nc.scalar.add(dt_[:], ps[:], bdw[:, j : j + 1])
                return dt_

            def conv_pe(lb, xtp):
                return [conv_pe_one(lb, j, xtp, j) for j in range(PE_CH)]

            def pointwise(lb, dwt_pe, dwtv, off):
                blk = BLKS[lb]
                ob = out_pool.tile([128, CCH, blk], f32, tag="outsb", name=f"ob{lb}")
                for dch in range(CCH):
                    po = out_psum.tile([128, blk], f32, tag="outps", name=f"po{dch}_{lb}")
                    for j in range(CCH):
                        rhs = (
                            dwt_pe[j][:, 0:blk]
                            if j < PE_CH
                            else dwtv[j - PE_CH][:, off : off + blk]
                        )
                        nc.tensor.matmul(
                            po[:],
                            lhsT=wpwt[:, j, dch, :],
                            rhs=rhs,
                            start=(j == 0),
                            stop=(j == CCH - 1),
                        )
                    nc.scalar.add(ob[:, dch, :], po[:], bpw[:, dch : dch + 1])
                # early blocks alternate the two HWDGE rings; late blocks go to
                # the SWDGE ring once its xp loads have drained
                if lb >= 6:
                    st = nc.gpsimd
                else:
                    st = (nc.sync, nc.scalar)[lb % 2]
                if lb >= 7:
                    st.dma_start(out=out_d[lb, :, 0:2, 0:blk], in_=ob[:, 0:2, :])
                    st.dma_start(out=out_d[lb, :, 2:4, 0:blk], in_=ob[:, 2:4, :])
                else:
                    st.dma_start(out=out_d[lb, :, :, 0:blk], in_=ob[:])

            for sb in range(NSB):
                xtv = load_dve(sb)
                lbs = [2 * sb, 2 * sb + 1]
                xtps = [load_pe(lb) for lb in lbs]
                dtv = conv_dve(sb, xtv)
                for lb, xtp in zip(lbs, xtps):
                    dwt_pe = conv_pe(lb, xtp)
                    pointwise(lb, dwt_pe, dtv, 512 * (lb - 2 * sb))

            # tail block (lb=8, 256 wide): all 4 chunks conv'd on PE
            lb = NBLK - 1
            xtp_t = xtp_pool.tile([128, PE_CH, PEW], f32r, tag="xtp", name="xtp_tail")
            nc.gpsimd.dma_start(out=xtp_t[:, :, 0:264], in_=xp_d[lb, :, :, 0:264])
            xq_t = xtp_pool.tile([128, NDV, 264], f32r, tag="xq", name="xq_t")
            nc.gpsimd.dma_start(out=xq_t[:], in_=xq_d[:])
            dwt_tail = [conv_pe_one(lb, j, xtp_t, j) for j in range(PE_CH)] + [
                conv_pe_one(lb, PE_CH + jj, xq_t, jj) for jj in range(NDV)
            ]
            blk = BLKS[lb]
            ob = out_pool.tile([128, CCH, blk], f32, tag="outsb", name="ob_tail")
            for dch in range(CCH):
                po = out_psum.tile([128, blk], f32, tag="outps", name=f"po{dch}_t")
                for j in range(CCH):
                    nc.tensor.matmul(
                        po[:],
                        lhsT=wpwt[:, j, dch, :],
                        rhs=dwt_tail[j][:, 0:blk],
                        start=(j == 0),
                        stop=(j == CCH - 1),
                    )
                nc.scalar.add(ob[:, dch, :], po[:], bpw[:, dch : dch + 1])
                if dch % 2 == 1:
                    nc.sync.dma_start(
                        out=out_d[lb, :, dch - 1 : dch + 1, 0:blk],
                        in_=ob[:, dch - 1 : dch + 1, :],
                    )

    nc.finalize()
    return nc


def _get_nc():
    if "nc" not in _cached:
        _cached["nc"] = _build_nc()
    return _cached["nc"]


def _analyze(segment_boundaries):
    starts = segment_boundaries[..., 0].astype(np.int64)  # [B,S]
    ends = segment_boundaries[..., 1].astype(np.int64)
    pos = np.arange(L)
    in_seg = (pos[None, None, :] >= starts[..., None]) & (
        pos[None, None, :] < ends[..., None]
    )  # [B,S,L]
    covered = in_seg.any(axis=1)
    seg_id = np.where(covered, in_seg.argmax(axis=1), -1)  # [B,L]
    return covered, seg_id


def _pack_views(slab):
    """slab: [C, SLAB_W] f32 (col 0..3 halo). Returns xp [NBLK,128,PE_CH,PEW]
    and xv [NSB,128,NDV,DVW] contiguous arrays."""
    xp = np.zeros((NBLK, 128, PE_CH, PEW), np.float32)
    for lb in range(NBLK):
        w = min(BLKS[lb] + 4, slab.shape[1] - 512 * lb)
        blkdat = slab[: PE_CH * 128, 512 * lb : 512 * lb + w]  # [256, w]
        xp[lb, :, :, :w] = blkdat.reshape(PE_CH, 128, w).transpose(1, 0, 2)
    xv = np.zeros((NSB, 128, NDV, DVW), np.float32)
    for sbi in range(NSB):
        w = min(SBS[sbi] + 4, slab.shape[1] - 1024 * sbi)
        blkdat = slab[PE_CH * 128 :, 1024 * sbi : 1024 * sbi + w]
        xv[sbi, :, :, :w] = blkdat.reshape(NDV, 128, w).transpose(1, 0, 2)
    xq = np.zeros((128, NDV, 264), np.float32)
    w = min(260, slab.shape[1] - 4096)
    blkdat = slab[PE_CH * 128 :, 4096 : 4096 + w]
    xq[:, :, :w] = blkdat.reshape(NDV, 128, w).transpose(1, 0, 2)
    return xp, xv, xq


def kernel(x, segment_boundaries, w_dw, b_dw, w_pw, b_pw):
    from concourse.bass_utils import run_bass_kernel_spmd

    x = np.asarray(x, dtype=np.float32)
    sb = np.asarray(segment_boundaries)
    w_dw = np.asarray(w_dw, dtype=np.float32)
    b_dw = np.asarray(b_dw, dtype=np.float32)
    w_pw = np.asarray(w_pw, dtype=np.float32)
    b_pw = np.asarray(b_pw, dtype=np.float32)

    covered, seg_id = _analyze(sb)

    # ---- run decomposition + stream build ----
    pieces = []
    src_b_parts = []
    src_l_parts = []
    run_start_of = np.full((B, L), -1, np.int64)
    for b in range(B):
        sid = seg_id[b]
        change = np.nonzero(np.diff(sid) != 0)[0] + 1
        bounds = np.concatenate([[0], change, [L]])
        for s, e in zip(bounds[:-1], bounds[1:]):
            if sid[s] < 0:
                continue
            run_start_of[b, s:e] = s
            pieces.append(np.zeros((4, C), np.float32))
            src_b_parts.append(np.full(4, -1, np.int64))
            src_l_parts.append(np.full(4, -1, np.int64))
            pieces.append(x[b, s:e])
            src_b_parts.append(np.full(e - s, b, np.int64))
            src_l_parts.append(np.arange(s, e, dtype=np.int64))
    if pieces:
        stream = np.concatenate(pieces, axis=0)
        src_b = np.concatenate(src_b_parts)
        src_l = np.concatenate(src_l_parts)
    else:
        stream = np.zeros((0, C), np.float32)
        src_b = np.zeros(0, np.int64)
        src_l = np.zeros(0, np.int64)
    T = stream.shape[0]
    Q = -(-T // NCORES) if T else 1
    assert Q + 4 <= OUT_ROWS, f"stream quota {Q} too large"

    # ---- per-core inputs ----
    wdiag = np.ascontiguousarray(
        w_dw.reshape(CCH, 128, K).transpose(1, 0, 2).reshape(128, CCH * K)
    )
    wpwt = np.ascontiguousarray(
        w_pw.reshape(CCH, 128, CCH, 128).transpose(3, 2, 0, 1)
    )
    bdwr = np.ascontiguousarray(b_dw.reshape(CCH, 128).T)
    bpwr = np.ascontiguousarray(b_pw.reshape(CCH, 128).T)
    cst = np.concatenate(
        [np.eye(128, dtype=np.float32), wdiag, bdwr, bpwr], axis=1
    )

    SLAB_W = 4 + OUT_ROWS + 4
    in_maps = []
    spans = []
    for i in range(NCORES):
        lo, hi = i * Q, min((i + 1) * Q, T)
        lo = min(lo, T)
        spans.append((lo, hi))
        buf = np.zeros((SLAB_W, C), np.float32)
        if hi > lo:
            hlo = max(0, lo - 4)
            buf[4 - (lo - hlo) : 4 + (hi - lo)] = stream[hlo:hi]
        xp, xv, xq = _pack_views(np.ascontiguousarray(buf.T))
        in_maps.append({"xp": xp, "xv": xv, "xq": xq, "cst": cst, "wpwt": wpwt})

    nc = _get_nc()
    res = run_bass_kernel_spmd(nc, in_maps, list(range(NCORES)))

    # ---- gather (device out is [NBLK, 128, CCH, 512] block-packed) ----
    so_out = np.zeros((T, C), np.float32)
    for i, (lo, hi) in enumerate(spans):
        if hi > lo:
            # [lb, q, h, r] -> [lb*512+r, h*128+q]
            full = res.results[i]["out"].transpose(0, 3, 2, 1).reshape(NBLK * 512, C)
            so_out[lo:hi] = full[: hi - lo]
    out = np.zeros((B, L, C), np.float32)
    mask = src_l >= 0
    out[src_b[mask], src_l[mask]] = so_out[mask]

    # ---- general-case sparse correction (pairwise mask vs run mask) ----
    need = []
    for d in range(1, K):
        m_ref = np.zeros((B, L), bool)
        m_ref[:, d:] = covered[:, d:] & (seg_id[:, d:] == seg_id[:, :-d])
        m_run = covered & (np.arange(L)[None, :] - run_start_of >= d)
        diff = m_ref.astype(np.int8) - m_run.astype(np.int8)
        if np.any(diff):
            bs, ls = np.nonzero(diff)
            need.append((d, bs, ls, diff[bs, ls].astype(np.float32)))
    if need:
        for d, bs, ls, sgn in need:
            xv_ = x[bs, ls - d, :]
            delta_dw = xv_ * w_dw[None, :, K - 1 - d] * sgn[:, None]
            out[bs, ls, :] += delta_dw @ w_pw.T

    return out
